# revision 1
# baseline (speedup 1.0000x reference)
"""Trainium2 Bass kernel for nn_CapsuleSequenceToGraph.

Strategy (8 NeuronCores, single SPMD NEFF):
  - Shard the sequence dim T across cores (weights are the dominant HBM
    traffic; T-sharding reads each weight byte exactly once chip-wide).
  - Inputs are pre-cast to bf16 on the host: halves DMA traffic and runs
    the PE at 1 cycle/row instead of fp32's 4.
  - Per core: pri = einsum('btj,tnjd->btnd') via PE matmuls, two t's packed
    per matmul with a block-diagonal x as the stationary operand.
    pri kept in SBUF as bf16, ONE contiguous tile per modality with layout
    [part=(t2,b=64), free=(pair,d,n)] so each routing pass is a single
    wide DVE instruction.
  - Dynamic routing (3 rounds + final readout):
      s_r = sum_t softmax_n(b_r) * pri   -> cross-core AllReduce of [B, n*d]
      v_r = tanh(s_r);  V_r = sum v_r    (running sum, bf16)
      b_{r+1} = sum_d V_r * pri          (fresh each round; b_0 = 0)
    Round 0's softmax over zeros is uniform, so s_0 = sum_t pri / 32 is
    computed on the PE with a stacked-identity selector + PSUM accumulation.
    All pri-sized elementwise multiplies use scalar_tensor_tensor, which
    supports the DVE 4x perf mode (4 elem/cycle for packed bf16 in SBUF);
    the d-reduction is a log2 tree of in-place strided STT adds (also 4x)
    instead of a 1 elem/cycle tensor_reduce.
    exp/tanh on the scalar engine.  |b| < 0.02 for these inputs so softmax
    needs no max-subtraction.
  - The final s_3 is NOT allreduced: each core emits its partial sum and the
    host reduces + applies tanh (saves one collective round per modality).
"""

import sys

if "/opt/trn_rl_repo" not in sys.path:
    sys.path.insert(0, "/opt/trn_rl_repo")

import numpy as np
import ml_dtypes

import concourse.bass as bass
import concourse.bacc as bacc
import concourse.mybir as mybir
from concourse import tile
from concourse.bass_utils import run_bass_kernel_spmd

F32 = mybir.dt.float32
BF16 = mybir.dt.bfloat16
AF = mybir.ActivationFunctionType
ALU = mybir.AluOpType

N_CORES = 8
B = 64
NV = 32  # n vertices
DC = 16  # capsule dim
J = 64  # MULT_D
T_DIMS = {"text": 128, "audio": 512, "video": 256, "frames": 256}
W_NAMES = {"text": "W_tpc", "audio": "W_apc", "video": "W_vpc", "frames": "W_fpc"}
# emit order: audio first — it is half the total work, so its (longest)
# dependency chain starts as early as possible and the small modalities
# fill engine gaps behind it
ORDER = ["audio", "video", "frames", "text"]
OUT_ORDER = ["text", "audio", "video", "frames"]
ROUNDS = 3
FN = DC * NV  # 512, free dim (d-major: flat = d*32 + n)

_CACHE = {}
AR_MODE = "cc"  # "cc" = per-mod AllReduce; "batch" = one AR per round;
                # "copy" = local bounce only (timing expt)
AR_BF16 = 1  # 1 = AllReduce payload in bf16 (halves collective bytes)
# Pool engine (gpsimd) runs at ~0.5 Gelem/s/partition vs DVE's 1.92 for bf16
# TT, so it takes ~1/3 of the multiply work to balance the two queues.
POOL_A = ("frames", "text")  # modalities whose A-mult runs on Pool
POOL_B = ("video", "text")  # modalities whose B-mult runs on Pool


def _pairs(mod):
    return T_DIMS[mod] // N_CORES // 2


def _build():
    nc = bacc.Bacc("TRN2", target_bir_lowering=False, debug=False, num_devices=N_CORES)

    xb_d = {}
    wr_d = {}
    out_d = {}
    for mod in ORDER:
        P = _pairs(mod)
        xb_d[mod] = nc.dram_tensor(f"xb_{mod}", [P, 128, 128], BF16, kind="ExternalInput")
        wr_d[mod] = nc.dram_tensor(f"wr_{mod}", [P, 128, FN], BF16, kind="ExternalInput")
        out_d[mod] = nc.dram_tensor(f"out_{mod}", [B, FN], F32, kind="ExternalOutput")
    sel_d = nc.dram_tensor("sel", [128, 64], BF16, kind="ExternalInput")

    rg = [list(range(N_CORES))]

    with tile.TileContext(nc) as tc:
        with (
            tc.tile_pool(name="io", bufs=4) as io,
            tc.tile_pool(name="pri", bufs=1) as pri_pool,
            tc.tile_pool(name="state", bufs=1) as st,
            tc.tile_pool(name="sm", bufs=1) as sm,
            tc.tile_pool(name="pp", bufs=2, space="PSUM") as ps_pri,
            tc.tile_pool(name="psacc", bufs=1, space="PSUM") as ps_s,
            tc.tile_pool(name="dram", bufs=1, space="DRAM") as dr,
        ):
            sel = st.tile([128, 64], BF16, tag="sel", name="sel")
            nc.sync.dma_start(sel[:], sel_d[:])

            pri = {}  # mod -> [128, P*FN] bf16 tile
            wk = {}  # mod -> [128, P*FN] bf16 scratch (w, then m)
            vvbf = {}  # mod -> [128, FN] bf16 (V duplicated in both halves)
            Vf = {}  # mod -> [64, FN] bf16 running sum of tanh
            bstate = {}  # mod -> [128, P*NV] bf16
            estate = {}  # mod -> [128, P*NV] bf16
            den = {}
            rinv = {}
            rcbf = {}
            s_glob = {}  # mod -> [64, FN] f32 allreduced s of current round

            def alloc_state(mod):
                P = _pairs(mod)
                pri[mod] = pri_pool.tile(
                    [128, P * FN], BF16, tag=f"pri_{mod}", name=f"pri_{mod}"
                )
                wk[mod] = pri_pool.tile(
                    [128, P * FN], BF16, tag=f"wk_{mod}", name=f"wk_{mod}"
                )
                vvbf[mod] = st.tile([128, FN], BF16, tag=f"vv_{mod}", name=f"vv_{mod}")
                Vf[mod] = st.tile([64, FN], BF16, tag=f"V_{mod}", name=f"V_{mod}")
                bstate[mod] = st.tile([128, P * NV], BF16, tag=f"b_{mod}", name=f"b_{mod}")
                estate[mod] = st.tile([128, P * NV], BF16, tag=f"e_{mod}", name=f"e_{mod}")
                den[mod] = st.tile([128, P], F32, tag=f"den_{mod}", name=f"den_{mod}")
                rinv[mod] = st.tile([128, P], F32, tag=f"ri_{mod}", name=f"ri_{mod}")
                rcbf[mod] = st.tile([128, P * NV], BF16, tag=f"rc_{mod}", name=f"rc_{mod}")

            ADT = BF16 if AR_BF16 else F32
            arb = {}  # round -> (batch_in, batch_out) DRAM tiles

            def emit_allreduce(mod, r, s_psum):
                """PSUM [64,FN] partial -> DRAM -> AllReduce -> SBUF."""
                s_loc = sm.tile([64, FN], ADT, tag=f"sl_{mod}", name=f"sl_{mod}")
                nc.scalar.copy(s_loc[:], s_psum[:])
                if AR_MODE == "batch":
                    mi = ORDER.index(mod)
                    if r not in arb:
                        bi = dr.tile([4 * 64, FN], ADT, tag=f"abi_{r}", name=f"abi_{r}")
                        bo = dr.tile([4 * 64, FN], ADT, tag=f"abo_{r}", name=f"abo_{r}")
                        arb[r] = (bi, bo)
                    bi, bo = arb[r]
                    nc.sync.dma_start(bi[mi * 64 : (mi + 1) * 64, :], s_loc[:])
                    if mi == len(ORDER) - 1:
                        nc.gpsimd.collective_compute(
                            "AllReduce",
                            ALU.add,
                            replica_groups=rg,
                            ins=[bi.opt()],
                            outs=[bo.opt()],
                        )
                        for mod2 in ORDER:
                            mj = ORDER.index(mod2)
                            sg = sm.tile([64, FN], ADT, tag=f"sg_{mod2}", name=f"sg_{mod2}")
                            nc.sync.dma_start(sg[:], bo[mj * 64 : (mj + 1) * 64, :])
                            s_glob[mod2] = sg
                    return
                ar_in = dr.tile([64, FN], ADT, tag=f"ari_{mod}_{r}", name=f"ari_{mod}_{r}")
                ar_out = dr.tile([64, FN], ADT, tag=f"aro_{mod}_{r}", name=f"aro_{mod}_{r}")
                nc.sync.dma_start(ar_in[:], s_loc[:])
                if AR_MODE == "cc":
                    nc.gpsimd.collective_compute(
                        "AllReduce",
                        ALU.add,
                        replica_groups=rg,
                        ins=[ar_in.opt()],
                        outs=[ar_out.opt()],
                    )
                else:
                    nc.sync.dma_start(ar_out[:], ar_in[:])
                sg = sm.tile([64, FN], ADT, tag=f"sg_{mod}", name=f"sg_{mod}")
                nc.sync.dma_start(sg[:], ar_out[:])
                s_glob[mod] = sg

            # ---------- phase 1: pri + s0 accumulation ----------
            for mod in ORDER:
                P = _pairs(mod)
                alloc_state(mod)
                # stream weights 4 pairs per DMA; drain PSUM 2 pairs per copy
                for g in range(P // 4):
                    xb_t = io.tile([128, 4 * 128], BF16, tag="xb", name="xb_t")
                    nc.sync.dma_start(
                        xb_t.rearrange("q (p j) -> q p j", p=4),
                        xb_d[mod][4 * g : 4 * g + 4].rearrange("p q j -> q p j"),
                    )
                    wr_t = io.tile([128, 4 * FN], BF16, tag="wr", name="wr_t")
                    nc.sync.dma_start(
                        wr_t.rearrange("q (p f) -> q p f", p=4),
                        wr_d[mod][4 * g : 4 * g + 4].rearrange("p q f -> q p f"),
                    )
                    for h in range(2):
                        pp = ps_pri.tile([128, 2 * FN], F32, tag="pp", name="pp")
                        for i in range(2):
                            p = 4 * g + 2 * h + i
                            nc.tensor.matmul(
                                pp[:, i * FN : (i + 1) * FN],
                                xb_t[:, (2 * h + i) * 128 : (2 * h + i + 1) * 128],
                                wr_t[:, (2 * h + i) * FN : (2 * h + i + 1) * FN],
                                start=True,
                                stop=True,
                            )
                        dst = pri[mod][:, (4 * g + 2 * h) * FN : (4 * g + 2 * h + 2) * FN]
                        if (2 * g + h) % 3 != 2:
                            nc.scalar.copy(dst, pp[:])
                        else:
                            nc.vector.tensor_copy(dst, pp[:])
                s_ps = ps_s.tile([64, FN], F32, tag=f"s_{mod}", name=f"s_{mod}")
                for p in range(P):
                    nc.tensor.matmul(
                        s_ps[:],
                        sel[:],
                        pri[mod][:, p * FN : (p + 1) * FN],
                        start=(p == 0),
                        stop=(p == P - 1),
                    )
                emit_allreduce(mod, 0, s_ps)

            # ---------- phase 2: routing rounds ----------
            def v_update_and_b(mod, r):
                """tanh(s_r) -> V (bf16); w = pri*V; b = sum_d w (STT tree)."""
                P = _pairs(mod)
                scale = (1.0 / NV) if r == 0 else 1.0
                t_bf = sm.tile([64, FN], BF16, tag=f"vt_{mod}", name=f"vt_{mod}")
                nc.scalar.activation(t_bf[:], s_glob[mod][:], AF.Tanh, scale=scale)
                if r == 0:
                    nc.vector.tensor_copy(Vf[mod][:], t_bf[:])
                else:
                    nc.vector.tensor_tensor(
                        out=Vf[mod][:], in0=Vf[mod][:], in1=t_bf[:], op=ALU.add
                    )
                nc.vector.tensor_copy(vvbf[mod][0:64, :], Vf[mod][:])
                nc.vector.tensor_copy(vvbf[mod][64:128, :], Vf[mod][:])
                w = wk[mod]
                pv = pri[mod].rearrange("q (p f) -> q p f", f=FN)
                wv = w.rearrange("q (p f) -> q p f", f=FN)
                # w = pri * VV  (one TT per modality, 2x DVE mode; a share of
                # the work goes to the otherwise-idle Pool engine)
                a_eng = nc.gpsimd if mod in POOL_A else nc.vector
                a_eng.tensor_tensor(
                    out=wv[:],
                    in0=pv[:],
                    in1=vvbf[mod].unsqueeze(1).broadcast_to([128, P, FN]),
                    op=ALU.mult,
                )
                # b = sum_d w: log2 tree of in-place contiguous adds (2x mode;
                # d-major layout makes each d-half a contiguous chunk)
                for h in (256, 128, 64):
                    nc.vector.tensor_tensor(
                        out=wv[:, :, 0:h],
                        in0=wv[:, :, 0:h],
                        in1=wv[:, :, h : 2 * h],
                        op=ALU.add,
                    )
                nc.vector.tensor_tensor(
                    out=bstate[mod].rearrange("q (p n) -> q p n", n=NV),
                    in0=wv[:, :, 0:NV],
                    in1=wv[:, :, NV : 2 * NV],
                    op=ALU.add,
                )
                # softmax over n (|b| << 1, no max subtraction needed)
                nc.scalar.activation(estate[mod][:], bstate[mod][:], AF.Exp)
                nc.vector.tensor_reduce(
                    out=den[mod][:],
                    in_=estate[mod].rearrange("q (p n) -> q p n", n=NV),
                    axis=mybir.AxisListType.X,
                    op=ALU.add,
                )
                nc.vector.reciprocal(rinv[mod][:], den[mod][:])
                nc.vector.tensor_tensor(
                    out=rcbf[mod].rearrange("q (p n) -> q p n", n=NV),
                    in0=estate[mod].rearrange("q (p n) -> q p n", n=NV),
                    in1=rinv[mod].unsqueeze(2).broadcast_to([128, P, NV]),
                    op=ALU.mult,
                )

            def mul1_and_s(mod, r):
                """m = rc * pri (reuses wk) ; s_psum = sum_t m via selector."""
                P = _pairs(mod)
                w = wk[mod]
                # m = rc * pri: one 4D TT per modality (2x DVE mode; some
                # modalities go to the Pool engine for queue balance)
                b_eng = nc.gpsimd if mod in POOL_B else nc.vector
                b_eng.tensor_tensor(
                    out=wk[mod].rearrange("q (p d n) -> q p d n", d=DC, n=NV),
                    in0=pri[mod].rearrange("q (p d n) -> q p d n", d=DC, n=NV),
                    in1=rcbf[mod]
                    .rearrange("q (p n) -> q p n", n=NV)
                    .unsqueeze(2)
                    .broadcast_to([128, P, DC, NV]),
                    op=ALU.mult,
                )
                s_ps = ps_s.tile([64, FN], F32, tag=f"s_{mod}", name=f"s_{mod}")
                for p in range(P):
                    nc.tensor.matmul(
                        s_ps[:],
                        sel[:],
                        w[:, p * FN : (p + 1) * FN],
                        start=(p == 0),
                        stop=(p == P - 1),
                    )
                return s_ps

            for r in range(ROUNDS):
                for mod in ORDER:
                    v_update_and_b(mod, r)
                    s_ps = mul1_and_s(mod, r + 1)
                    if r < ROUNDS - 1:
                        emit_allreduce(mod, r + 1, s_ps)
                    else:
                        s_out = sm.tile([64, FN], F32, tag=f"sl_{mod}", name=f"so_{mod}")
                        nc.scalar.copy(s_out[:], s_ps[:])
                        nc.sync.dma_start(out_d[mod][:], s_out[:])

    nc.compile()
    return nc


def _host_prep(inputs):
    """Build the 8 per-core input maps (T-sharded, PE-ready layouts)."""
    sel = np.concatenate([np.eye(64, dtype=np.float32)] * 2, axis=0).astype(
        ml_dtypes.bfloat16
    )
    in_maps = []
    for c in range(N_CORES):
        m = {"sel": sel}
        for mod in ORDER:
            T = T_DIMS[mod]
            Tc = T // N_CORES
            P = Tc // 2
            t0 = c * Tc
            x = np.asarray(inputs[mod], dtype=np.float32)  # [B, T, J]
            W = np.asarray(inputs[W_NAMES[mod]], dtype=np.float32)  # [T,NV,J,DC]
            xs = np.ascontiguousarray(
                x[:, t0 : t0 + Tc, :].transpose(1, 2, 0)
            )  # [Tc, J, B]
            xb = np.zeros((P, 128, 128), dtype=ml_dtypes.bfloat16)
            xb[:, 0:64, 0:64] = xs[0::2]
            xb[:, 64:128, 64:128] = xs[1::2]
            wt = W[t0 : t0 + Tc].transpose(0, 2, 3, 1).reshape(Tc, J, FN)
            # wt[t, j, d*32+n] = W[t, n, j, d]
            wr = np.empty((P, 128, FN), dtype=ml_dtypes.bfloat16)
            wr[:, 0:64, :] = wt[0::2]
            wr[:, 64:128, :] = wt[1::2]
            m[f"xb_{mod}"] = xb
            m[f"wr_{mod}"] = np.ascontiguousarray(wr)
        in_maps.append(m)
    return in_maps


def _gather(results):
    outs = []
    for mod in OUT_ORDER:
        s = np.zeros((B, FN), dtype=np.float64)
        for c in range(N_CORES):
            s += np.asarray(results[c][f"out_{mod}"], dtype=np.float64)
        o = np.tanh(s.astype(np.float32))
        outs.append(np.ascontiguousarray(o.reshape(B, DC, NV).transpose(0, 2, 1)))
    return tuple(outs)


def kernel(**inputs):
    if "nc" not in _CACHE:
        _CACHE["nc"] = _build()
    nc = _CACHE["nc"]
    in_maps = _host_prep(inputs)
    res = run_bass_kernel_spmd(nc, in_maps, core_ids=list(range(N_CORES)))
    return _gather(res.results)

